# revision 1
# baseline (speedup 1.0000x reference)
"""Trainium2 Bass kernel for the DinMod LSTM+CfC (NCP) recurrent network.

Strategy:
  - Data-parallel over 8 NeuronCores: batch 64 -> 8 per core, weights replicated.
  - Phase A (parallel): fc1 projection feats = x @ fc1_w.T + b, then the
    time-invariant input projections of the LSTM cell and CfC layer 0 are
    precomputed for all T steps as big matmuls (transposed layout: feature
    dim on partitions, (t, b) on the free dim).
  - Phase B (sequential scan over T=512): tiny per-step recurrent cell with a
    minimized critical path:
      * precomputed per-step terms are injected into PSUM via identity
        matmuls (off the critical path) and the recurrent matmuls accumulate
        on top (start=False),
      * sigmoid for the CfC gate is 0.5*tanh(0.5x)+0.5 with the 0.5 folded
        into weights, so each CfC layer needs a single Tanh activation op,
      * elementwise gate algebra is packed into quadrant-aligned "stacked"
        tiles so one vector op covers two gates,
      * the CfC state hl = 0.5*(f1 + f2 + t*(f2-f1)) is never materialized:
        t*(f2-f1) is written into spare rows of the F tile and every
        consumer contracts F directly with host-precomposed weights.
  - All operand blocks live at SBUF partition bases in {0, 32, 64, 96}
    (quadrant addressing); dual-SBUF vector ops use equal bases, f2-f1 is
    built on the tensor engine (PSUM) so the p-multiply is mixed-space.
  - Every compute instruction carries at most one new semaphore wait
    (hardware limit): absorber 1x1 matmuls observe each DMA once, injects
    are pinned behind the previous step's Cd matmul, and bacc's
    generate_event_semaphores splits anything left.

Layout conventions (partition ranges):
  P_Y/Y (97p):  sig(fg) 0:33 | sig(ig) 64:97
  P_AO (97p):   og 0:33 | ia 64:97
  X (97p):      c 0:33  | tanh(ia) 64:97
  P_l/F_l (CfC layer l, k outputs): f1 0:k | f2 32:32+k | t 64:64+k | pt 96:96+k
"""

import numpy as np

import concourse.bass as bass
import concourse.mybir as mybir
from concourse import bacc
from concourse.tile import TileContext
from concourse.tile_rust import add_dep_helper
from concourse.bass_utils import run_bass_kernel_spmd

IN_DIM, LATENT = 512, 256
INTER, COMMAND, MOTOR = 18, 12, 3
STATE = INTER + COMMAND + MOTOR  # 33
B, T_FULL, N_CORES = 64, 512, 8
BS = B // N_CORES  # 8

F32 = mybir.dt.float32
AF = mybir.ActivationFunctionType


def _gpos(j):
    """state index (0..32) -> gapped partition position."""
    if j < INTER:
        return j
    if j < INTER + COMMAND:
        return 32 + (j - INTER)
    return 64 + (j - INTER - COMMAND)


def prep_weights(inp):
    """Fold/transpose all model weights into device layouts. numpy f32."""
    g = {k: np.asarray(v, np.float32) for k, v in inp.items()}
    w = {}
    w["fc1T"] = np.ascontiguousarray(g["fc1_w"].T)             # (512, 256)
    fb = np.zeros((128, 2), np.float32)
    fb[:, 0] = g["fc1_b"][:128]
    fb[:, 1] = g["fc1_b"][128:]
    w["fc1b"] = fb

    wi, bi, wh = g["lstm_wi"], g["lstm_bi"], g["lstm_wh"]
    ia, ig, fg, og = (slice(0, 33), slice(33, 66), slice(66, 99), slice(99, 132))

    def pack97(rows_lo, rows_hi, src, axis_cols):
        """Build (axis_cols, 97) matrix: cols 0:33 <- src[rows_lo], 64:97 <- src[rows_hi]."""
        m = np.zeros((axis_cols, 97), np.float32)
        m[:, 0:33] = src[rows_lo].T
        m[:, 64:97] = src[rows_hi].T
        return m

    w["wiT_Y"] = pack97(fg, ig, wi, LATENT)                     # (256, 97)
    w["wiT_AO"] = pack97(og, ia, wi, LATENT)
    bY = np.zeros((97, 1), np.float32)
    bY[0:33, 0] = bi[fg] + 1.0
    bY[64:97, 0] = bi[ig]
    w["biasY"] = bY
    bAO = np.zeros((97, 1), np.float32)
    bAO[0:33, 0] = bi[og]
    bAO[64:97, 0] = bi[ia]
    w["biasAO"] = bAO

    # LSTM recurrent weights.  h_state = concat(hl_0, hl_1, hl_2) and each
    # hl_l = s_l + p_l with s_l = Ms_l @ [f1;f2].  We never materialize h:
    # consumers contract directly against [F_l (with p packed at rows 96:)],
    # with lhsT rows 0:2k = Ms_l.T @ W_block, rows 96:96+k = W_block.
    # hl_l = 0.5*(f1 + f2 + pt_l) where pt_l = t*(f2-f1) sits at rows 96: of
    # the F_l tile; consumers contract F_l directly with [0.5W; 0.5W; 0.5W].
    ks = [INTER, COMMAND, MOTOR]
    koff = [0, INTER, INTER + COMMAND]
    for bi_, k in enumerate(ks):
        for pname, lo_sl, hi_sl in (("Y", fg, ig), ("AO", og, ia)):
            blk = np.zeros((k, 97), np.float32)   # raw W rows for this state block
            for j in range(k):
                blk[j, 0:33] = wh[lo_sl, koff[bi_] + j]
                blk[j, 64:97] = wh[hi_sl, koff[bi_] + j]
            comb = np.zeros((96 + k, 97), np.float32)
            comb[0:k, :] = 0.5 * blk
            comb[32:32 + k, :] = 0.5 * blk
            comb[96:96 + k, :] = 0.5 * blk
            w[f"wh{pname}{bi_}"] = comb

    # CfC layers
    dims = [(LATENT, INTER), (INTER, COMMAND), (COMMAND, MOTOR)]
    for l, (p_l, k) in enumerate(dims):
        w1m = g[f"ff1w{l}"] * g[f"mask{l}"]
        w2m = g[f"ff2w{l}"] * g[f"mask{l}"]
        wab = 0.5 * (g[f"taw{l}"] + g[f"tbw{l}"])
        bti = 0.5 * (g[f"tab{l}"] + g[f"tbb{l}"])
        nrows = 64 + k  # gate rows: f1 0:k | f2 32:32+k | t 64:64+k
        bias = np.zeros((nrows, 1), np.float32)
        bias[0:k, 0] = g[f"ff1b{l}"]
        bias[32:32 + k, 0] = g[f"ff2b{l}"]
        bias[64:64 + k, 0] = bti
        w[f"bias{l}"] = bias

        def pack_cols(col_sel, in_rows, row_map):
            # gate-row layout (quadrant per role): f1 0:k | f2 32:32+k | t 64:64+k
            m = np.zeros((in_rows, nrows), np.float32)
            for jj, rr in row_map:
                m[rr, 0:k] = w1m[:, col_sel][:, jj]
                m[rr, 32:32 + k] = w2m[:, col_sel][:, jj]
                m[rr, 64:64 + k] = wab[:, col_sel][:, jj]
            return m

        if l == 0:
            # input part (from feats, 256) and recurrent part (inter slice)
            w["W0inT"] = pack_cols(slice(0, LATENT), LATENT,
                                   [(jj, jj) for jj in range(LATENT)])
            w["W0recT"] = pack_cols(slice(LATENT, LATENT + k), INTER,
                                    [(jj, jj) for jj in range(INTER)])
        elif l == 1:
            # input = hl0 = 0.5*(f1 + f2 + pt0), contracted against F0ext
            w1in = pack_cols(slice(0, INTER), INTER,
                             [(jj, jj) for jj in range(INTER)])      # (18, 76)
            comb = np.zeros((96 + INTER, nrows), np.float32)
            comb[0:INTER, :] = 0.5 * w1in
            comb[32:32 + INTER, :] = 0.5 * w1in
            comb[96:96 + INTER, :] = 0.5 * w1in
            w["W1comb"] = comb
            w["W1recT"] = pack_cols(slice(INTER, INTER + k), STATE,
                                    [(jj, INTER + jj) for jj in range(COMMAND)])
        else:
            # input = hl1 = 0.5*(f1 + f2 + pt1), contracted against F1ext
            w2in = pack_cols(slice(0, COMMAND), COMMAND,
                             [(jj, jj) for jj in range(COMMAND)])    # (12, 67)
            comb = np.zeros((96 + COMMAND, nrows), np.float32)
            comb[0:COMMAND, :] = 0.5 * w2in
            comb[32:32 + COMMAND, :] = 0.5 * w2in
            comb[96:96 + COMMAND, :] = 0.5 * w2in
            w["W2comb"] = comb
            w["W2recT"] = pack_cols(slice(COMMAND, COMMAND + k), STATE,
                                    [(jj, STATE - MOTOR + jj) for jj in range(MOTOR)])

    bg0 = np.zeros((64 + INTER, 1), np.float32)
    bg0[0:INTER, 0] = g["ff1b0"]
    bg0[32:32 + INTER, 0] = g["ff2b0"]
    bg0[64:64 + INTER, 0] = 0.5 * (g["tab0"] + g["tbb0"])
    w["biasg0"] = bg0

    # constant matrices for tensor-engine linear combos
    cc = np.zeros((97, 33), np.float32)
    for j in range(33):
        cc[j, j] = 1.0
        cc[64 + j, j] = 1.0
    w["Cc"] = cc
    # s2 = 0.5*(f1 + f2) for the motor output (hl2 = s2 + 0.5*pt2)
    c = np.zeros((32 + MOTOR, MOTOR), np.float32)
    for j in range(MOTOR):
        c[j, j] = 0.5
        c[32 + j, j] = 0.5
    w["C2"] = c
    # d_l = f2 - f1 on the tensor engine (PSUM out, so the p-multiply reads
    # one PSUM operand — dual-SBUF TensorTensor requires equal base partitions)
    for l, k in [(0, INTER), (1, COMMAND), (2, MOTOR)]:
        c = np.zeros((32 + k, k), np.float32)
        for j in range(k):
            c[j, j] = -1.0
            c[32 + j, j] = 1.0
        w[f"Cd{l}"] = c
    i97 = np.zeros((97, 97), np.float32)
    for r in list(range(33)) + list(range(64, 97)):
        i97[r, r] = 1.0
    w["I97"] = i97
    i82 = np.zeros((82, 82), np.float32)
    for r in (list(range(INTER)) + list(range(32, 32 + INTER))
              + list(range(64, 64 + INTER))):
        i82[r, r] = 1.0
    w["I82"] = i82
    return w


# DRAM input specs (name -> shape) besides xt
def _weight_specs(T):
    return {
        "fc1T": (512, 256), "fc1b": (128, 2),
        "wiT_Y": (256, 97), "wiT_AO": (256, 97),
        "biasY": (97, 1), "biasAO": (97, 1),
        "whY0": (114, 97), "whY1": (108, 97), "whY2": (99, 97),
        "whAO0": (114, 97), "whAO1": (108, 97), "whAO2": (99, 97),
        "W0inT": (256, 82), "W0recT": (18, 82), "biasg0": (82, 1),
        "W1comb": (114, 76), "W1recT": (33, 76), "bias1": (76, 1),
        "W2comb": (108, 67), "W2recT": (33, 67), "bias2": (67, 1),
        "Cc": (97, 33), "C2": (35, 3),
        "Cd0": (50, 18), "Cd1": (44, 12), "Cd2": (35, 3),
        "I97": (97, 97), "I82": (82, 82),
    }


def build_program(T=T_FULL, opts=()):
    """Build the Bass program for one core: xt (512, T*BS) -> out (3, T*BS)."""
    opts = set(opts)
    NF = T * BS
    nc = bacc.Bacc("TRN2")
    xt_d = nc.dram_tensor("xt", [IN_DIM, NF], F32, kind="ExternalInput")
    wd = {}
    for nm, shp in _weight_specs(T).items():
        wd[nm] = nc.dram_tensor(nm, list(shp), F32, kind="ExternalInput")
    out_d = nc.dram_tensor("out", [MOTOR, NF], F32, kind="ExternalOutput")

    NCH = NF // 512 if NF >= 512 else 1   # free-dim chunks for phase A
    CH = min(512, NF)

    with TileContext(nc) as tc:
        with tc.tile_pool(name="wpool", bufs=1) as wp, \
             tc.tile_pool(name="data", bufs=1) as dp:
            # ---- load weights (tensors with >128 rows are partition-chunked) ----
            sb = {}
            for nm, shp in _weight_specs(T).items():
                rows, cols = shp
                if rows > 128:
                    nch = (rows + 127) // 128
                    t = wp.tile([128, nch, cols], F32, tag=f"w_{nm}")
                    # single DMA per tensor so consumers wait on one queue only
                    nc.sync.dma_start(
                        out=t, in_=wd[nm].rearrange("(c p) n -> p c n", p=128))
                else:
                    t = wp.tile([rows, cols], F32, tag=f"w_{nm}")
                    nc.sync.dma_start(out=t, in_=wd[nm][:, :])
                sb[nm] = t

            # ---- load x (transposed on host): 4 chunks of 128 partitions ----
            xt_sb = dp.tile([128, 4, NF], F32)
            nc.sync.dma_start(out=xt_sb,
                              in_=xt_d.rearrange("(c p) n -> p c n", p=128))

            feats = dp.tile([128, 2, NF], F32)
            zinY = dp.tile([97, NF], F32)
            zinAO = dp.tile([97, NF], F32)
            g0in = dp.tile([82, NF], F32)
            out_sb = dp.tile([MOTOR, NF], F32)

            # ---- Phase A: big parallel matmuls ----
            with tc.tile_pool(name="pa", bufs=2, space="PSUM") as pa:
                # Wait-absorbers: the fused LDWEIGHTS+MATMUL can carry only one
                # semaphore wait, so have the PE observe every DMA-loaded tile
                # once via a 1x1 dummy matmul (one producer each, disjoint
                # PSUM columns so no write-ordering self-wait is added);
                # real matmuls then need at most one new wait.
                srcs = list(sb.values()) + [xt_sb]
                absorb = pa.tile([1, len(srcs) + 3], F32, tag="absorb")
                for j, t_ in enumerate(srcs):
                    a2 = t_[0:1, 0, 0:1] if len(t_.shape) == 3 else t_[0:1, 0:1]
                    nc.tensor.matmul(absorb[:, j:j + 1], a2, a2, start=True, stop=True)
                for m in range(2):
                    for n in range(NCH):
                        ps = pa.tile([128, CH], F32)
                        for k in range(4):
                            nc.tensor.matmul(
                                ps,
                                sb["fc1T"][:, k, 128 * m:128 * (m + 1)],
                                xt_sb[:, k, n * CH:(n + 1) * CH],
                                start=(k == 0), stop=(k == 3),
                            )
                        nc.scalar.activation(
                            feats[:, m, n * CH:(n + 1) * CH], ps,
                            AF.Identity, bias=sb["fc1b"][:, m:m + 1],
                        )
                for tgt, lhs, bias_nm, rows in (
                    (zinY, "wiT_Y", "biasY", 97),
                    (zinAO, "wiT_AO", "biasAO", 97),
                    (g0in, "W0inT", "biasg0", 82),
                ):
                    for n in range(NCH):
                        ps = pa.tile([128, CH], F32)
                        for kk in range(2):
                            nc.tensor.matmul(
                                ps[0:rows, :],
                                sb[lhs][:, kk, :],
                                feats[:, kk, n * CH:(n + 1) * CH],
                                start=(kk == 0), stop=(kk == 1),
                            )
                        nc.scalar.activation(
                            tgt[:, n * CH:(n + 1) * CH], ps[0:rows, :],
                            AF.Identity, bias=sb[bias_nm][:, 0:1],
                        )
                # absorb the first zin-chunk ACT waits so the t=0 injects
                # carry a single semaphore wait
                zin_anchor = None
                for j, tgt in enumerate((zinY, zinAO, g0in)):
                    a2 = tgt[0:1, NF - 1:NF]   # last chunk -> max ACT sem value
                    zin_anchor = nc.tensor.matmul(
                        absorb[:, len(srcs) + j:len(srcs) + j + 1],
                        a2, a2, start=True, stop=True)

            # ---- Phase B: the scan ----
            with tc.tile_pool(name="st", bufs=1) as stp, \
                 tc.tile_pool(name="sc", bufs=3) as scp, \
                 tc.tile_pool(name="pY", bufs=1, space="PSUM") as pY, \
                 tc.tile_pool(name="pAO", bufs=1, space="PSUM") as pAO, \
                 tc.tile_pool(name="pC", bufs=1, space="PSUM") as pC, \
                 tc.tile_pool(name="p0", bufs=1, space="PSUM") as p0p, \
                 tc.tile_pool(name="p1", bufs=1, space="PSUM") as p1p, \
                 tc.tile_pool(name="p2", bufs=1, space="PSUM") as p2p, \
                 tc.tile_pool(name="pDS", bufs=1, space="PSUM") as pDS:

                # ping-pong state tiles; rows outside the written blocks are
                # memset to zero once and never rewritten
                xA = stp.tile([97, BS], F32, tag="xA")
                xB = stp.tile([97, BS], F32, tag="xB")
                f0A = stp.tile([96 + INTER, BS], F32, tag="f0A")
                f0B = stp.tile([96 + INTER, BS], F32, tag="f0B")
                f1A = stp.tile([96 + COMMAND, BS], F32, tag="f1A")
                f1B = stp.tile([96 + COMMAND, BS], F32, tag="f1B")
                f2A = stp.tile([96 + MOTOR, BS], F32, tag="f2A")
                f2B = stp.tile([96 + MOTOR, BS], F32, tag="f2B")
                for t_ in (xA, xB, f0A, f0B, f1A, f1B, f2A, f2B):
                    nc.vector.memset(t_, 0.0)
                x_pair = (xA, xB)
                f_pairs = ((f0A, f0B), (f1A, f1B), (f2A, f2B))
                prev_anchor = zin_anchor  # last Cd matmul of the previous step

                scan_reps = 1
                for o in opts:
                    if isinstance(o, str) and o.startswith("reps"):
                        scan_reps = int(o[4:])
                # experiment flag: 3 extra dead PE matmuls per step, to test
                # whether the measured execute path is PE-instruction-bound
                pe3 = "pe3" in opts
                steps = [(rep, t) for rep in range(scan_reps) for t in range(T)]
                for rep, t in steps:
                    x_cur = x_pair[t % 2]
                    x_next = x_pair[(t + 1) % 2]
                    F0p, F1p, F2p = (fp[(t + 1) % 2] for fp in f_pairs)   # prev step
                    F0, F1, F2 = (fp[t % 2] for fp in f_pairs)            # this step
                    c0, c1 = t * BS, (t + 1) * BS

                    # --- LSTM gates: zin inject + recurrent part contracted
                    #     against the previous step's F tiles (h never formed) ---
                    PY = pY.tile([97, BS], F32)
                    PA = pAO.tile([97, BS], F32)
                    last = (t == 0)
                    miY = nc.tensor.matmul(PY, sb["I97"], zinY[:, c0:c1], start=True, stop=last)
                    miA = nc.tensor.matmul(PA, sb["I97"], zinAO[:, c0:c1], start=True, stop=last)
                    # keep injects behind the previous step's last Cd matmul in
                    # PE order so their WAR-on-ACT wait is already subsumed
                    # (the fused LDWEIGHTS can carry only one sem wait)
                    if prev_anchor is not None:
                        add_dep_helper(miY.ins, prev_anchor.ins, sync=False,
                                       reason="inject after prev Cd (wait budget)")
                        add_dep_helper(miA.ins, prev_anchor.ins, sync=False,
                                       reason="inject after prev Cd (wait budget)")
                    if t > 0:
                        for i, (Fprev, kb) in enumerate(((F0p, INTER), (F1p, COMMAND), (F2p, MOTOR))):
                            lastb = (i == 2)
                            nc.tensor.matmul(PY, sb[f"whY{i}"], Fprev[:, :],
                                             start=False, stop=lastb)
                            nc.tensor.matmul(PA, sb[f"whAO{i}"], Fprev[:, :],
                                             start=False, stop=lastb)

                    Y = scp.tile([97, BS], F32, tag="Y")
                    nc.scalar.activation(Y, PY, AF.Sigmoid)               # sig(fg)|sig(ig)
                    nc.scalar.activation(x_cur[64:97, :], PA[64:97, :], AF.Tanh)  # tanh(ia)
                    O = scp.tile([33, BS], F32, tag="O")
                    nc.scalar.activation(O, PA[0:33, :], AF.Sigmoid)      # sig(og)

                    S = scp.tile([97, BS], F32, tag="S")
                    nc.vector.tensor_mul(S, x_cur, Y)                     # c*sfg | T_ia*sig
                    PCt = pC.tile([33, BS], F32)
                    nc.tensor.matmul(PCt, sb["Cc"], S, start=True, stop=True)  # c_new
                    Tc = scp.tile([33, BS], F32, tag="Tc")
                    nc.scalar.activation(Tc, PCt, AF.Tanh)
                    hl = scp.tile([33, BS], F32, tag="hl")
                    nc.vector.tensor_mul(hl, Tc, O)                       # h_lstm

                    if pe3:
                        dmy = pDS.tile([3, BS], F32, tag="DS")
                        for _ in range(3):
                            nc.tensor.matmul(dmy, sb["I97"][0:3, 0:3],
                                             zinY[0:3, c0:c1], start=True, stop=True)

                    # --- CfC layer 0 ---
                    P0 = p0p.tile([82, BS], F32)
                    mi0 = nc.tensor.matmul(P0, sb["I82"], g0in[:, c0:c1], start=True, stop=False)
                    if prev_anchor is not None:
                        add_dep_helper(mi0.ins, prev_anchor.ins, sync=False,
                                       reason="inject after prev Cd (wait budget)")
                    nc.tensor.matmul(P0, sb["W0recT"], hl[0:18, :], start=False, stop=True)
                    nc.scalar.activation(F0[0:82, :], P0, AF.Tanh)
                    # carry c for the next step: emitted after F0 so the copy
                    # lands in the ScalarE idle window instead of delaying F0
                    # (it is only needed by the next step's S-multiply)
                    nc.scalar.copy(x_next[0:33, :], PCt)
                    D0 = pDS.tile([INTER, BS], F32, tag="DSd")
                    nc.tensor.matmul(D0, sb["Cd0"], F0[0:32 + INTER, :], start=True, stop=True)
                    nc.vector.tensor_mul(F0[96:96 + INTER, :], F0[64:64 + INTER, :], D0)

                    # --- CfC layer 1 ---
                    P1 = p1p.tile([76, BS], F32)
                    nc.tensor.matmul(P1, sb["W1recT"], hl[0:33, :], start=True, stop=False)
                    nc.tensor.matmul(P1, sb["W1comb"], F0[:, :], start=False, stop=True)
                    nc.scalar.activation(F1[0:76, :], P1, AF.Tanh, bias=sb["bias1"][:, 0:1])
                    D1 = pDS.tile([COMMAND, BS], F32, tag="DSd")
                    nc.tensor.matmul(D1, sb["Cd1"], F1[0:32 + COMMAND, :], start=True, stop=True)
                    nc.vector.tensor_mul(F1[96:96 + COMMAND, :], F1[64:64 + COMMAND, :], D1)

                    # --- CfC layer 2 ---
                    P2 = p2p.tile([67, BS], F32)
                    nc.tensor.matmul(P2, sb["W2recT"], hl[0:33, :], start=True, stop=False)
                    nc.tensor.matmul(P2, sb["W2comb"], F1[:, :], start=False, stop=True)
                    nc.scalar.activation(F2[0:67, :], P2, AF.Tanh, bias=sb["bias2"][:, 0:1])
                    D2 = pDS.tile([MOTOR, BS], F32, tag="DSd")
                    prev_anchor = nc.tensor.matmul(
                        D2, sb["Cd2"], F2[0:32 + MOTOR, :], start=True, stop=True)
                    nc.vector.tensor_mul(F2[96:96 + MOTOR, :], F2[64:64 + MOTOR, :], D2)
                    # motor output hl2 = s2 + 0.5*pt2 (off the critical chain)
                    DS2 = pDS.tile([MOTOR, BS], F32, tag="DS")
                    nc.tensor.matmul(DS2, sb["C2"], F2[0:32 + MOTOR, :], start=True, stop=True)
                    nc.vector.scalar_tensor_tensor(
                        out_sb[:, c0:c1], F2[96:96 + MOTOR, :], 0.5, DS2,
                        mybir.AluOpType.mult, mybir.AluOpType.add)

            nc.sync.dma_start(out=out_d[:, :], in_=out_sb[:, :])
    nc.compile()   # bacc passes: split multi-waits into event semaphores etc.
    return nc


def host_prep(inputs, T=T_FULL):
    """Shard + transpose x per core; fold weights (shared)."""
    x = np.asarray(inputs["x"], np.float32)
    w = prep_weights(inputs)
    in_maps = []
    for i in range(N_CORES):
        xs = x[i * BS:(i + 1) * BS, :T, :]                  # (BS, T, 512)
        xt = np.ascontiguousarray(xs.transpose(2, 1, 0).reshape(IN_DIM, T * BS))
        m = {"xt": xt}
        m.update(w)
        in_maps.append(m)
    return in_maps


def gather_output(results, T=T_FULL):
    outs = []
    for i in range(N_CORES):
        o = np.asarray(results[i]["out"])                   # (3, T*BS)
        outs.append(o.reshape(MOTOR, T, BS).transpose(2, 1, 0))  # (BS, T, 3)
    return np.concatenate(outs, axis=0)


_PROGRAM_CACHE = {}


def kernel(**inputs):
    T = T_FULL
    if T not in _PROGRAM_CACHE:
        _PROGRAM_CACHE[T] = build_program(T)
    nc = _PROGRAM_CACHE[T]
    in_maps = host_prep(inputs, T)
    res = run_bass_kernel_spmd(nc, in_maps, list(range(N_CORES)))
    return gather_output(res.results, T)



# revision 2
# speedup vs baseline: 3.2316x; 3.2316x over previous
"""Trainium2 Bass kernel for DinMod LSTM+CfC via parallel-in-time iteration.

Key idea: replace the T=512 sequential scan (~33 instructions/step, ~17k
instructions) with 4 full-trajectory sweeps (~1.1k instructions total):

  sweep m:  z_t   = zin_t + Wh @ h^{m-1}_{t-1}          (big matmuls, all t)
            a_t   = sigmoid(fg_t + 1); b_t = tanh(ia_t) * sigmoid(ig_t)
            c_t   = a_t * c_{t-1} + b_t                  (EXACT via HW
                                                          tensor_tensor_scan)
            hL_t  = tanh(c_t) * sigmoid(og_t)
            h^m_t = CfC(feats_t, hL_t)                   (big matmuls, all t)

The LSTM c-recurrence is linear given the gates, so it is solved exactly
per sweep by the DVE scan instruction; the remaining h-feedback contracts
~10x per sweep (measured in fp32: sweep 3 -> 1.8e-3, sweep 4 -> 2.2e-4
max-rel vs sequential; tolerance is 2e-2).

Column layout: col = 513*b + 1 + t for batch-lane b (8 per core), step t;
col 513*b is a zero pad. The pad makes the scan reset state between lanes
(state = 0*state + 0) and provides the h_{-1} = 0 boundary for the
shifted (t-1) reads. F-trajectory tiles carry one extra 513-col zero
front block so the shift-by-one-col read never goes out of bounds.

Quadrant packing (partition bases 0/32/64/96) as in the sequential
baseline: PE matmuls re-inject precomputed input projections and
accumulate recurrent parts; gate nonlinearities are chunked (PSUM free
cap 512); everything else is full-width single instructions.
"""

import numpy as np

import concourse.bass as bass
import concourse.mybir as mybir
from concourse import bacc
from concourse.tile import TileContext
from concourse.bass_utils import run_bass_kernel_spmd

IN_DIM, LATENT = 512, 256
INTER, COMMAND, MOTOR = 18, 12, 3
STATE = INTER + COMMAND + MOTOR  # 33
B, T_FULL, N_CORES = 64, 512, 8
BS = B // N_CORES  # 8

BLK = T_FULL + 1        # 513: per-lane block (1 pad col + T data cols)
NCOL = BS * BLK         # 4104
FPAD = BLK              # front zero block width of F tiles
FCOL = FPAD + NCOL      # 4617
CH = 456                # free-dim chunk (PSUM cap 512); 9 * 456 = 4104
NCH = NCOL // CH        # 9
SWEEPS = 3

F32 = mybir.dt.float32
AF = mybir.ActivationFunctionType
ALU = mybir.AluOpType

ia_sl, ig_sl = slice(0, 33), slice(33, 66)
fg_sl, og_sl = slice(66, 99), slice(99, 132)


def prep_weights(inp):
    g = {k: np.asarray(v, np.float32) for k, v in inp.items()}
    w = {}
    fc1_w, fc1_b = g["fc1_w"], g["fc1_b"]
    wi, bi, wh = g["lstm_wi"], g["lstm_bi"], g["lstm_wh"]

    weff = wi @ fc1_w                      # (132, 512)
    beff = wi @ fc1_b + bi                 # (132,)

    def gate_pair(lo, hi, bias_lo_extra=0.0):
        m = np.zeros((IN_DIM, 97), np.float32)
        m[:, 0:33] = weff[lo].T
        m[:, 64:97] = weff[hi].T
        bv = np.zeros((97, 1), np.float32)
        bv[0:33, 0] = beff[lo] + bias_lo_extra
        bv[64:97, 0] = beff[hi]
        return m, bv

    w["WXY"], w["biasY"] = gate_pair(fg_sl, ig_sl, 1.0)   # [sfg | sig]
    # og gate via tanh: sig(x) = 0.5*tanh(x/2) + 0.5; the 0.5 logit factor
    # is folded here, the output 0.5/+0.5 into the hL stt and W*recT scales
    w["WXA"], w["biasA"] = gate_pair(og_sl, ia_sl)        # [og' | tia]
    w["WXA"][:, 0:33] *= 0.5
    w["biasA"][0:33] *= 0.5

    # CfC per-layer masked weights
    w1m, w2m, wab, b1v, b2v, btv = [], [], [], [], [], []
    for l in range(3):
        w1m.append(g[f"ff1w{l}"] * g[f"mask{l}"])
        w2m.append(g[f"ff2w{l}"] * g[f"mask{l}"])
        wab.append(0.5 * (g[f"taw{l}"] + g[f"tbw{l}"]))
        b1v.append(g[f"ff1b{l}"])
        b2v.append(g[f"ff2b{l}"])
        btv.append(0.5 * (g[f"tab{l}"] + g[f"tbb{l}"]))

    # L0 input projection through fc1 (feats never materialized on device)
    wx0 = np.zeros((IN_DIM, 82), np.float32)
    bs0 = np.zeros((82, 1), np.float32)
    for qoff, wm, bb in ((0, w1m[0], b1v[0]), (32, w2m[0], b2v[0]),
                         (64, wab[0], btv[0])):
        wx0[:, qoff:qoff + INTER] = (wm[:, :LATENT] @ fc1_w).T
        bs0[qoff:qoff + INTER, 0] = wm[:, :LATENT] @ fc1_b + bb
    w["WX0"], w["bias0"] = wx0, bs0

    w["I97"] = np.eye(97, dtype=np.float32)
    w["I82"] = np.eye(82, dtype=np.float32)

    # recurrent (hL) projections into each layer's gate quadrants
    def rec_mat(l, p_lo, k, nout):
        m = np.zeros((STATE, nout), np.float32)
        for qoff, wm in ((0, w1m[l]), (32, w2m[l]), (64, wab[l])):
            m[p_lo:p_lo + k, qoff:qoff + k] = wm[:, -k:].T
        return m

    # 0.5x: the hL buffer holds 2*hL = tanh(c)*(tanh(og/2)+1)
    w["W0recT"] = 0.5 * rec_mat(0, 0, INTER, 82)
    w["W1recT"] = 0.5 * rec_mat(1, INTER, COMMAND, 76)
    w["W2recT"] = 0.5 * rec_mat(2, INTER + COMMAND, MOTOR, 67)

    # comb: previous layer's F tile (f1@0:k, f2@32.., pt@96..) -> this layer
    # hl_prev = 0.5*(f1 + f2 + pt)
    def comb_mat(l, kp, nrows, nout, k):
        m = np.zeros((nrows, nout), np.float32)
        for jj in range(kp):
            for r in (jj, 32 + jj, 96 + jj):
                m[r, 0:k] = 0.5 * w1m[l][:, jj]
                m[r, 32:32 + k] = 0.5 * w2m[l][:, jj]
                m[r, 64:64 + k] = 0.5 * wab[l][:, jj]
        return m

    w["W1comb"] = comb_mat(1, INTER, 114, 76, COMMAND)
    w["W2comb"] = comb_mat(2, COMMAND, 108, 67, MOTOR)

    bias1 = np.zeros((76, 1), np.float32)
    bias1[0:12, 0], bias1[32:44, 0], bias1[64:76, 0] = b1v[1], b2v[1], btv[1]
    w["bias1"] = bias1
    bias2 = np.zeros((67, 1), np.float32)
    bias2[0:3, 0], bias2[32:35, 0], bias2[64:67, 0] = b1v[2], b2v[2], btv[2]
    w["bias2"] = bias2

    # f2 - f1 selectors
    for l, k in ((0, INTER), (1, COMMAND), (2, MOTOR)):
        m = np.zeros((32 + k, k), np.float32)
        for j in range(k):
            m[j, j] = -1.0
            m[32 + j, j] = 1.0
        w[f"Cd{l}"] = m

    # motor output: hl2 = 0.5*(f1 + f2 + pt)
    c2 = np.zeros((99, 3), np.float32)
    for j in range(MOTOR):
        c2[j, j] = 0.5
        c2[32 + j, j] = 0.5
        c2[96 + j, j] = 0.5
    w["C2full"] = c2

    # LSTM recurrent: gates from F tiles (h = concat of hl_l = 0.5*(f1+f2+pt))
    koff = [0, INTER, INTER + COMMAND]
    for nm, lo, hi in (("WHY", fg_sl, ig_sl), ("WHA", og_sl, ia_sl)):
        wlo, whi = wh[lo], wh[hi]          # (33, 33) each
        lo_scale = 0.25 if nm == "WHA" else 0.5   # og' logit is halved
        for l, k in ((0, INTER), (1, COMMAND), (2, MOTOR)):
            nr = [114, 108, 99][l]
            m = np.zeros((nr, 97), np.float32)
            for jj in range(k):
                j = koff[l] + jj
                for r in (jj, 32 + jj, 96 + jj):
                    m[r, 0:33] = lo_scale * wlo[:, j]
                    m[r, 64:97] = 0.5 * whi[:, j]
            w[f"{nm}{l}"] = m
    return w


def _weight_specs():
    return {
        "WXY": (512, 97), "WXA": (512, 97), "WX0": (512, 82),
        "biasY": (97, 1), "biasA": (97, 1), "bias0": (82, 1),
        "I97": (97, 97), "I82": (82, 82),
        "W0recT": (33, 82), "W1recT": (33, 76), "W2recT": (33, 67),
        "W1comb": (114, 76), "W2comb": (108, 67),
        "bias1": (76, 1), "bias2": (67, 1),
        "Cd0": (50, 18), "Cd1": (44, 12), "Cd2": (35, 3),
        "C2full": (99, 3),
        "WHY0": (114, 97), "WHY1": (108, 97), "WHY2": (99, 97),
        "WHA0": (114, 97), "WHA1": (108, 97), "WHA2": (99, 97),
    }


def build_program(T=T_FULL, opts=()):
    opts = set(opts)
    reps = 1
    sweeps = SWEEPS
    for o in opts:
        if isinstance(o, str) and o.startswith("reps"):
            reps = int(o[4:])
        if isinstance(o, str) and o.startswith("sweeps"):
            sweeps = int(o[6:])

    nc = bacc.Bacc("TRN2")
    xt_d = nc.dram_tensor("xt", [128, 4, NCOL], F32, kind="ExternalInput")
    wd = {}
    for nm, shp in _weight_specs().items():
        wd[nm] = nc.dram_tensor(nm, list(shp), F32, kind="ExternalInput")
    out_d = nc.dram_tensor("out", [MOTOR, NCOL], F32, kind="ExternalOutput")

    with TileContext(nc) as tc:
        with tc.tile_pool(name="wp", bufs=1) as wp, \
             tc.tile_pool(name="dp", bufs=1) as dp:
            sb = {}
            for nm, shp in _weight_specs().items():
                rows, cols = shp
                if rows > 128:
                    nch = (rows + 127) // 128
                    t = wp.tile([128, nch, cols], F32, tag=f"w_{nm}")
                    nc.sync.dma_start(
                        out=t, in_=wd[nm].rearrange("(c p) n -> p c n", p=128))
                else:
                    t = wp.tile([rows, cols], F32, tag=f"w_{nm}")
                    nc.sync.dma_start(out=t, in_=wd[nm][:, :])
                sb[nm] = t

            # persistent trajectory buffers
            zinY = dp.tile([97, NCOL], F32)
            zinA = dp.tile([97, NCOL], F32)
            g0in = dp.tile([82, NCOL], F32)
            SG = dp.tile([97, NCOL], F32)    # [a=sig(fg+1)@0:33 | sig(ig)@64:97]
            G2 = dp.tile([97, NCOL], F32)    # [sig(og)@0:33 | tanh(ia)@64:97,
                                             #  then scan-out c_t @64:97]
            Bt = dp.tile([33, NCOL], F32)    # b-term, then reused for tanh(c)
            hLb = dp.tile([33, NCOL], F32)   # LSTM h_t
            F0T = dp.tile([114, FCOL], F32)  # f1@0:18|f2@32:50|t@64:82|pt@96:114
            F1T = dp.tile([108, FCOL], F32)
            F2T = dp.tile([99, FCOL], F32)
            ost = dp.tile([MOTOR, CH], F32)  # out staging per chunk

            for t_ in (F0T, F1T, F2T):
                nc.vector.memset(t_, 0.0)

            # ---- Phase A: project zinY/zinA/g0in from x (through fc1) ----
            with tc.tile_pool(name="xp", bufs=2) as xp, \
                 tc.tile_pool(name="pa", bufs=1, space="PSUM") as pa:
                for c in range(NCH):
                    J = slice(c * CH, (c + 1) * CH)
                    xt_c = xp.tile([128, 4, CH], F32)
                    nc.sync.dma_start(out=xt_c, in_=xt_d[:, :, J])
                    # each target also emits sweep-0's activated gates so the
                    # first sweep skips its gate loop entirely
                    for tgt, lhs, bnm, rows, g0 in (
                            (zinY, "WXY", "biasY", 97, ("SG", AF.Sigmoid)),
                            (zinA, "WXA", "biasA", 97, ("G2", AF.Tanh)),
                            (g0in, "WX0", "bias0", 82, None)):
                        psf = pa.tile([97, CH], F32, tag="pa")
                        ps = psf[0:rows, :]
                        for k in range(4):
                            nc.tensor.matmul(ps, sb[lhs][:, k, 0:rows],
                                             xt_c[:, k, :],
                                             start=(k == 0), stop=(k == 3))
                        nc.scalar.activation(tgt[:, J], ps, AF.Identity,
                                             bias=sb[bnm][:, 0:1])
                        if g0 is not None:
                            gt = SG if g0[0] == "SG" else G2
                            nc.scalar.activation(gt[:, J], ps, g0[1],
                                                 bias=sb[bnm][:, 0:1])

            SGv = SG.rearrange("p (b c) -> p b c", c=BLK)
            Btv = Bt.rearrange("p (b c) -> p b c", c=BLK)
            F0v = F0T.rearrange("p (b c) -> p b c", c=BLK)
            F1v = F1T.rearrange("p (b c) -> p b c", c=BLK)
            F2v = F2T.rearrange("p (b c) -> p b c", c=BLK)

            # ---- sweeps ----
            with tc.tile_pool(name="pG", bufs=2, space="PSUM") as pGp, \
                 tc.tile_pool(name="pC", bufs=2, space="PSUM") as pCp, \
                 tc.tile_pool(name="pD", bufs=2, space="PSUM") as pDp:
                for rep in range(reps):
                    for s in range(sweeps):
                        first = (rep == 0 and s == 0)
                        last = (rep == reps - 1 and s == sweeps - 1)
                        # loop-1: LSTM gates (sweep 0's come from phase A)
                        for c in range(NCH) if not first else ():
                            J = slice(c * CH, (c + 1) * CH)
                            Jm = slice(FPAD - 1 + c * CH, FPAD - 1 + (c + 1) * CH)
                            PY = pGp.tile([97, CH], F32, tag="G")
                            PA = pGp.tile([97, CH], F32, tag="G")
                            for P, zin, r0, r1, r2 in (
                                    (PY, zinY, "WHY0", "WHY1", "WHY2"),
                                    (PA, zinA, "WHA0", "WHA1", "WHA2")):
                                nc.tensor.matmul(P, sb["I97"], zin[:, J],
                                                 start=True, stop=False)
                                nc.tensor.matmul(P, sb[r0], F0T[:, Jm],
                                                 start=False, stop=False)
                                nc.tensor.matmul(P, sb[r1], F1T[:, Jm],
                                                 start=False, stop=False)
                                nc.tensor.matmul(P, sb[r2], F2T[:, Jm],
                                                 start=False, stop=True)
                            nc.scalar.activation(SG[:, J], PY, AF.Sigmoid)
                            nc.scalar.activation(G2[:, J], PA, AF.Tanh)
                        # full-width block: exact c-scan, hL
                        nc.vector.memset(SGv[0:33, :, 0:1], 0.0)
                        nc.vector.tensor_mul(Bt, G2[64:97, :], SG[64:97, :])
                        nc.vector.memset(Btv[:, :, 0:1], 0.0)
                        nc.vector.tensor_tensor_scan(
                            G2[64:97, :], SG[0:33, :], Bt, 0.0,
                            ALU.mult, ALU.add)                     # c_t
                        nc.scalar.activation(Bt, G2[64:97, :], AF.Tanh)
                        # hLb = 2*hL = tanh(c) * (tanh(og/2) + 1); the 0.5 is
                        # folded into W0recT/W1recT/W2recT
                        nc.vector.scalar_tensor_tensor(
                            hLb, G2[0:33, :], 1.0, Bt, ALU.add, ALU.mult)
                        # loop-2: CfC chain
                        for c in range(NCH):
                            J = slice(c * CH, (c + 1) * CH)
                            Jw = slice(FPAD + c * CH, FPAD + (c + 1) * CH)
                            P0 = pCp.tile([82, CH], F32, tag="P")
                            nc.tensor.matmul(P0, sb["I82"], g0in[:, J],
                                             start=True, stop=False)
                            nc.tensor.matmul(P0, sb["W0recT"], hLb[:, J],
                                             start=False, stop=True)
                            nc.scalar.activation(F0T[0:82, Jw], P0, AF.Tanh)
                            D0 = pDp.tile([INTER, CH], F32, tag="D")
                            nc.tensor.matmul(D0, sb["Cd0"], F0T[0:50, Jw],
                                             start=True, stop=True)
                            nc.vector.tensor_mul(F0T[96:114, Jw],
                                                 F0T[64:82, Jw], D0)
                            P1f = pCp.tile([82, CH], F32, tag="P")
                            P1 = P1f[0:76, :]
                            nc.tensor.matmul(P1, sb["W1comb"], F0T[0:114, Jw],
                                             start=True, stop=False)
                            nc.tensor.matmul(P1, sb["W1recT"], hLb[:, J],
                                             start=False, stop=True)
                            nc.scalar.activation(F1T[0:76, Jw], P1, AF.Tanh,
                                                 bias=sb["bias1"][:, 0:1])
                            D1f = pDp.tile([INTER, CH], F32, tag="D")
                            D1 = D1f[0:COMMAND, :]
                            nc.tensor.matmul(D1, sb["Cd1"], F1T[0:44, Jw],
                                             start=True, stop=True)
                            nc.vector.tensor_mul(F1T[96:108, Jw],
                                                 F1T[64:76, Jw], D1)
                            P2f = pCp.tile([82, CH], F32, tag="P")
                            P2 = P2f[0:67, :]
                            nc.tensor.matmul(P2, sb["W2comb"], F1T[0:108, Jw],
                                             start=True, stop=False)
                            nc.tensor.matmul(P2, sb["W2recT"], hLb[:, J],
                                             start=False, stop=True)
                            nc.scalar.activation(F2T[0:67, Jw], P2, AF.Tanh,
                                                 bias=sb["bias2"][:, 0:1])
                            D2f = pDp.tile([INTER, CH], F32, tag="D")
                            D2 = D2f[0:MOTOR, :]
                            nc.tensor.matmul(D2, sb["Cd2"], F2T[0:35, Jw],
                                             start=True, stop=True)
                            nc.vector.tensor_mul(F2T[96:99, Jw],
                                                 F2T[64:67, Jw], D2)
                            if last:
                                POf = pDp.tile([INTER, CH], F32, tag="D")
                                PO = POf[0:MOTOR, :]
                                nc.tensor.matmul(PO, sb["C2full"],
                                                 F2T[0:99, Jw],
                                                 start=True, stop=True)
                                nc.scalar.activation(ost, PO, AF.Identity)
                                nc.sync.dma_start(out=out_d[:, J], in_=ost)
                        if not last:
                            nc.vector.memset(F0v[:, :, 0:1], 0.0)
                            nc.vector.memset(F1v[:, :, 0:1], 0.0)
                            nc.vector.memset(F2v[:, :, 0:1], 0.0)
    nc.compile()
    return nc


def host_prep(inputs, T=T_FULL):
    x = np.asarray(inputs["x"], np.float32)
    w = prep_weights(inputs)
    in_maps = []
    for i in range(N_CORES):
        xs = x[i * BS:(i + 1) * BS, :T, :]          # (BS, T, 512)
        xt = np.zeros((IN_DIM, BS, BLK), np.float32)
        xt[:, :, 1:] = xs.transpose(2, 0, 1)
        xt = xt.reshape(4, 128, NCOL).transpose(1, 0, 2)   # (128, 4, NCOL)
        m = {"xt": np.ascontiguousarray(xt)}
        m.update(w)
        in_maps.append(m)
    return in_maps


def gather_output(results, T=T_FULL):
    outs = []
    for i in range(N_CORES):
        o = np.asarray(results[i]["out"]).reshape(MOTOR, BS, BLK)
        outs.append(o[:, :, 1:].transpose(1, 2, 0))  # (BS, T, 3)
    return np.concatenate(outs, axis=0)


_PROGRAM_CACHE = {}


def kernel(**inputs):
    T = T_FULL
    if T not in _PROGRAM_CACHE:
        _PROGRAM_CACHE[T] = build_program(T)
    nc = _PROGRAM_CACHE[T]
    in_maps = host_prep(inputs, T)
    res = run_bass_kernel_spmd(nc, in_maps, list(range(N_CORES)))
    return gather_output(res.results, T)


# revision 7
# speedup vs baseline: 3.6219x; 1.1208x over previous
"""Trainium2 Bass kernel for DinMod LSTM+CfC via parallel-in-time iteration.

Key idea: replace the T=512 sequential scan (~33 instructions/step, ~17k
instructions) with 3 full-trajectory sweeps (~650 instructions total):

  sweep m:  z_t   = zin_t + Wh @ h^{m-1}_{t-1}          (big matmuls, all t)
            a_t   = sigmoid(fg_t + 1); b_t = tanh(ia_t) * sigmoid(ig_t)
            c_t   = a_t * c_{t-1} + b_t                  (EXACT via HW
                                                          tensor_tensor_scan)
            hL_t  = tanh(c_t) * sigmoid(og_t)
            h^m_t = CfC(feats_t, hL_t)                   (big matmuls, all t)

The LSTM c-recurrence is linear given the gates, so it is solved exactly
per sweep by the DVE scan instruction; the remaining h-feedback contracts
~10x per sweep (measured in fp32: sweep 3 -> 1.8e-3, sweep 4 -> 2.2e-4
max-rel vs sequential; tolerance is 2e-2).

Column layout: col = 512*b + t for batch-lane b (8 per core), step t.
The scan resets between lanes by forcing a = sig(fg+1) to 0 at each
lane's t=0 col (c_0 = b_0 exactly since c_{-1} = 0). F-trajectory tiles
carry a 512-col zero front block so the shifted (t-1) reads never go out
of bounds, and each lane's LAST col is re-zeroed between sweeps (h_T is
never a valid h_{t-1}; the next lane's t=0 shifted read must see 0).

Quadrant packing (partition bases 0/32/64/96) as in the sequential
baseline. The execute path's cost is dominated by PE instruction count
(~78us/matmul measured), so precomputed input projections are added on
the vector engine (scalar_tensor_tensor on PSUM) instead of identity
re-inject matmuls, and chunks are exactly one PSUM bank (512 cols).
"""

import numpy as np

import concourse.bass as bass
import concourse.mybir as mybir
from concourse import bacc
from concourse.tile import TileContext
from concourse.bass_utils import run_bass_kernel_spmd

IN_DIM, LATENT = 512, 256
INTER, COMMAND, MOTOR = 18, 12, 3
STATE = INTER + COMMAND + MOTOR  # 33
B, T_FULL, N_CORES = 64, 512, 8
BS = B // N_CORES  # 8

BLK = T_FULL            # 512: per-lane block (t = 0..511, no pad cols)
NCOL = BS * BLK         # 4096
FPAD = BLK              # front zero block width of F tiles
FCOL = FPAD + NCOL      # 4608
CH = 512                # free-dim chunk = one PSUM bank exactly; 8 chunks
NCH = NCOL // CH        # 8
SWEEPS = 3

F32 = mybir.dt.float32
AF = mybir.ActivationFunctionType
ALU = mybir.AluOpType

ia_sl, ig_sl = slice(0, 33), slice(33, 66)
fg_sl, og_sl = slice(66, 99), slice(99, 132)


def prep_weights(inp):
    g = {k: np.asarray(v, np.float32) for k, v in inp.items()}
    w = {}
    fc1_w, fc1_b = g["fc1_w"], g["fc1_b"]
    wi, bi, wh = g["lstm_wi"], g["lstm_bi"], g["lstm_wh"]

    weff = wi @ fc1_w                      # (132, 512)
    beff = wi @ fc1_b + bi                 # (132,)

    def gate_pair(lo, hi, bias_lo_extra=0.0):
        m = np.zeros((IN_DIM, 97), np.float32)
        m[:, 0:33] = weff[lo].T
        m[:, 64:97] = weff[hi].T
        bv = np.zeros((97, 1), np.float32)
        bv[0:33, 0] = beff[lo] + bias_lo_extra
        bv[64:97, 0] = beff[hi]
        return m, bv

    w["WXY"], w["biasY"] = gate_pair(fg_sl, ig_sl, 1.0)   # [sfg | sig]
    # og gate via tanh: sig(x) = 0.5*tanh(x/2) + 0.5; the 0.5 logit factor
    # is folded here, the output 0.5/+0.5 into the hL stt and W*recT scales
    w["WXA"], w["biasA"] = gate_pair(og_sl, ia_sl)        # [og' | tia]
    w["WXA"][:, 0:33] *= 0.5
    w["biasA"][0:33] *= 0.5

    # CfC per-layer masked weights
    w1m, w2m, wab, b1v, b2v, btv = [], [], [], [], [], []
    for l in range(3):
        w1m.append(g[f"ff1w{l}"] * g[f"mask{l}"])
        w2m.append(g[f"ff2w{l}"] * g[f"mask{l}"])
        wab.append(0.5 * (g[f"taw{l}"] + g[f"tbw{l}"]))
        b1v.append(g[f"ff1b{l}"])
        b2v.append(g[f"ff2b{l}"])
        btv.append(0.5 * (g[f"tab{l}"] + g[f"tbb{l}"]))

    # L0 input projection through fc1 (feats never materialized on device)
    wx0 = np.zeros((IN_DIM, 82), np.float32)
    bs0 = np.zeros((82, 1), np.float32)
    for qoff, wm, bb in ((0, w1m[0], b1v[0]), (32, w2m[0], b2v[0]),
                         (64, wab[0], btv[0])):
        wx0[:, qoff:qoff + INTER] = (wm[:, :LATENT] @ fc1_w).T
        bs0[qoff:qoff + INTER, 0] = wm[:, :LATENT] @ fc1_b + bb
    w["WX0"], w["bias0"] = wx0, bs0

    # recurrent (hL) projections into each layer's gate quadrants
    def rec_mat(l, p_lo, k, nout):
        m = np.zeros((STATE, nout), np.float32)
        for qoff, wm in ((0, w1m[l]), (32, w2m[l]), (64, wab[l])):
            m[p_lo:p_lo + k, qoff:qoff + k] = wm[:, -k:].T
        return m

    # 0.5x: the hL buffer holds 2*hL = tanh(c)*(tanh(og/2)+1)
    w["W0recT"] = 0.5 * rec_mat(0, 0, INTER, 82)
    w["W1recT"] = 0.5 * rec_mat(1, INTER, COMMAND, 76)
    w["W2recT"] = 0.5 * rec_mat(2, INTER + COMMAND, MOTOR, 67)

    # comb: previous layer's F tile (f1@0:k, f2@32.., pt@96..) -> this layer
    # hl_prev = 0.5*(f1 + f2 + pt)
    def comb_mat(l, kp, nrows, nout, k):
        m = np.zeros((nrows, nout), np.float32)
        for jj in range(kp):
            for r in (jj, 32 + jj, 96 + jj):
                m[r, 0:k] = 0.5 * w1m[l][:, jj]
                m[r, 32:32 + k] = 0.5 * w2m[l][:, jj]
                m[r, 64:64 + k] = 0.5 * wab[l][:, jj]
        return m

    w["W1comb"] = comb_mat(1, INTER, 114, 76, COMMAND)
    w["W2comb"] = comb_mat(2, COMMAND, 108, 67, MOTOR)

    bias1 = np.zeros((76, 1), np.float32)
    bias1[0:12, 0], bias1[32:44, 0], bias1[64:76, 0] = b1v[1], b2v[1], btv[1]
    w["bias1"] = bias1
    bias2 = np.zeros((67, 1), np.float32)
    bias2[0:3, 0], bias2[32:35, 0], bias2[64:67, 0] = b1v[2], b2v[2], btv[2]
    w["bias2"] = bias2

    # f2 - f1 selectors
    for l, k in ((0, INTER), (1, COMMAND), (2, MOTOR)):
        m = np.zeros((32 + k, k), np.float32)
        for j in range(k):
            m[j, j] = -1.0
            m[32 + j, j] = 1.0
        w[f"Cd{l}"] = m

    # motor output: hl2 = 0.5*(f1 + f2 + pt)
    c2 = np.zeros((99, 3), np.float32)
    for j in range(MOTOR):
        c2[j, j] = 0.5
        c2[32 + j, j] = 0.5
        c2[96 + j, j] = 0.5
    w["C2full"] = c2

    # LSTM recurrent: gates from F tiles (h = concat of hl_l = 0.5*(f1+f2+pt))
    koff = [0, INTER, INTER + COMMAND]
    for nm, lo, hi in (("WHY", fg_sl, ig_sl), ("WHA", og_sl, ia_sl)):
        wlo, whi = wh[lo], wh[hi]          # (33, 33) each
        lo_scale = 0.25 if nm == "WHA" else 0.5   # og' logit is halved
        for l, k in ((0, INTER), (1, COMMAND), (2, MOTOR)):
            nr = [114, 108, 99][l]
            m = np.zeros((nr, 97), np.float32)
            for jj in range(k):
                j = koff[l] + jj
                for r in (jj, 32 + jj, 96 + jj):
                    m[r, 0:33] = lo_scale * wlo[:, j]
                    m[r, 64:97] = 0.5 * whi[:, j]
            w[f"{nm}{l}"] = m
    return w


def _weight_specs():
    return {
        "WXY": (512, 97), "WXA": (512, 97), "WX0": (512, 82),
        "biasY": (97, 1), "biasA": (97, 1), "bias0": (82, 1),
        "W0recT": (33, 82), "W1recT": (33, 76), "W2recT": (33, 67),
        "W1comb": (114, 76), "W2comb": (108, 67),
        "bias1": (76, 1), "bias2": (67, 1),
        "Cd0": (50, 18), "Cd1": (44, 12), "Cd2": (35, 3),
        "C2full": (99, 3),
        "WHY0": (114, 97), "WHY1": (108, 97), "WHY2": (99, 97),
        "WHA0": (114, 97), "WHA1": (108, 97), "WHA2": (99, 97),
    }


def build_program(T=T_FULL, opts=()):
    opts = set(opts)
    reps = 1
    sweeps = SWEEPS
    for o in opts:
        if isinstance(o, str) and o.startswith("reps"):
            reps = int(o[4:])
        if isinstance(o, str) and o.startswith("sweeps"):
            sweeps = int(o[6:])

    dmm = dfw = 0
    for o in opts:
        if isinstance(o, str) and o.startswith("dmm"):
            dmm = int(o[3:])       # dummy small matmuls per rep (calibration)
        if isinstance(o, str) and o.startswith("dfw"):
            dfw = int(o[3:])       # dummy full-width DVE ops per rep

    nc = bacc.Bacc("TRN2")
    xt_d = nc.dram_tensor("xt", [128, 4, NCOL], F32, kind="ExternalInput")
    wd = {}
    for nm, shp in _weight_specs().items():
        wd[nm] = nc.dram_tensor(nm, list(shp), F32, kind="ExternalInput")
    out_d = nc.dram_tensor("out", [MOTOR, NCOL], F32, kind="ExternalOutput")

    with TileContext(nc) as tc:
        with tc.tile_pool(name="wp", bufs=1) as wp, \
             tc.tile_pool(name="dp", bufs=1) as dp:
            sb = {}
            for nm, shp in _weight_specs().items():
                rows, cols = shp
                if rows > 128:
                    nch = (rows + 127) // 128
                    t = wp.tile([128, nch, cols], F32, tag=f"w_{nm}")
                    nc.sync.dma_start(
                        out=t, in_=wd[nm].rearrange("(c p) n -> p c n", p=128))
                else:
                    t = wp.tile([rows, cols], F32, tag=f"w_{nm}")
                    nc.sync.dma_start(out=t, in_=wd[nm][:, :])
                sb[nm] = t

            # persistent trajectory buffers
            zinY = dp.tile([97, NCOL], F32)
            zinA = dp.tile([97, NCOL], F32)
            g0in = dp.tile([82, NCOL], F32)
            SG = dp.tile([97, NCOL], F32)    # [a=sig(fg+1)@0:33 | sig(ig)@64:97]
            G2 = dp.tile([97, NCOL], F32)    # [sig(og)@0:33 | tanh(ia)@64:97,
                                             #  then scan-out c_t @64:97]
            Bt = dp.tile([33, NCOL], F32)    # b-term, then reused for tanh(c)
            hLb = dp.tile([33, NCOL], F32)   # LSTM h_t
            F0T = dp.tile([114, FCOL], F32)  # f1@0:18|f2@32:50|t@64:82|pt@96:114
            F1T = dp.tile([108, FCOL], F32)
            F2T = dp.tile([99, FCOL], F32)
            ost = dp.tile([MOTOR, CH], F32)  # out staging per chunk

            for t_ in (F0T, F1T, F2T):
                nc.vector.memset(t_, 0.0)

            # ---- Phase A: project zinY/zinA/g0in from x (through fc1) ----
            with tc.tile_pool(name="xp", bufs=2) as xp, \
                 tc.tile_pool(name="pa", bufs=1, space="PSUM") as pa:
                for c in range(NCH):
                    J = slice(c * CH, (c + 1) * CH)
                    xt_c = xp.tile([128, 4, CH], F32)
                    nc.sync.dma_start(out=xt_c, in_=xt_d[:, :, J])
                    # each target also emits sweep-0's activated gates so the
                    # first sweep skips its gate loop entirely
                    for tgt, lhs, bnm, rows, g0 in (
                            (zinY, "WXY", "biasY", 97, ("SG", AF.Sigmoid)),
                            (zinA, "WXA", "biasA", 97, ("G2", AF.Tanh)),
                            (g0in, "WX0", "bias0", 82, None)):
                        psf = pa.tile([97, CH], F32, tag="pa")
                        ps = psf[0:rows, :]
                        for k in range(4):
                            nc.tensor.matmul(ps, sb[lhs][:, k, 0:rows],
                                             xt_c[:, k, :],
                                             start=(k == 0), stop=(k == 3))
                        nc.scalar.activation(tgt[:, J], ps, AF.Identity,
                                             bias=sb[bnm][:, 0:1])
                        if g0 is not None:
                            gt = SG if g0[0] == "SG" else G2
                            nc.scalar.activation(gt[:, J], ps, g0[1],
                                                 bias=sb[bnm][:, 0:1])

            SGv = SG.rearrange("p (b c) -> p b c", c=BLK)
            F0v = F0T.rearrange("p (b c) -> p b c", c=BLK)
            F1v = F1T.rearrange("p (b c) -> p b c", c=BLK)
            F2v = F2T.rearrange("p (b c) -> p b c", c=BLK)

            # ---- sweeps ----
            with tc.tile_pool(name="pG", bufs=2, space="PSUM") as pGp, \
                 tc.tile_pool(name="pC", bufs=2, space="PSUM") as pCp, \
                 tc.tile_pool(name="pD", bufs=2, space="PSUM") as pDp, \
                 tc.tile_pool(name="sp", bufs=3) as spp:
                for rep in range(reps):
                    for s in range(sweeps):
                        first = (rep == 0 and s == 0)
                        last = (rep == reps - 1 and s == sweeps - 1)
                        # loop-1: LSTM gates (sweep 0's come from phase A)
                        for c in range(NCH) if not first else ():
                            J = slice(c * CH, (c + 1) * CH)
                            Jm = slice(FPAD - 1 + c * CH, FPAD - 1 + (c + 1) * CH)
                            PY = pGp.tile([97, CH], F32, tag="G")
                            PA = pGp.tile([97, CH], F32, tag="G")
                            for P, zin, r0, r1, r2, tgt, fn in (
                                    (PY, zinY, "WHY0", "WHY1", "WHY2",
                                     SG, AF.Sigmoid),
                                    (PA, zinA, "WHA0", "WHA1", "WHA2",
                                     G2, AF.Tanh)):
                                nc.tensor.matmul(P, sb[r0], F0T[:, Jm],
                                                 start=True, stop=False)
                                nc.tensor.matmul(P, sb[r1], F1T[:, Jm],
                                                 start=False, stop=False)
                                nc.tensor.matmul(P, sb[r2], F2T[:, Jm],
                                                 start=False, stop=True)
                                # add the precomputed input part on the DVE,
                                # keeping the PE free for the next matmul
                                Gt = spp.tile([97, CH], F32, tag="t97")
                                nc.vector.scalar_tensor_tensor(
                                    Gt, P, 1.0, zin[:, J], ALU.mult, ALU.add)
                                nc.scalar.activation(tgt[:, J], Gt, fn)
                        # full-width block: exact c-scan, hL.
                        # a=sig(fg+1) is forced to 0 at each lane's t=0 col,
                        # which makes the scan compute c_0 = b_0 exactly
                        # (c_{-1}=0) and resets state between lanes.
                        nc.vector.memset(SGv[0:33, :, 0:1], 0.0)
                        nc.vector.tensor_mul(Bt, G2[64:97, :], SG[64:97, :])
                        nc.vector.tensor_tensor_scan(
                            G2[64:97, :], SG[0:33, :], Bt, 0.0,
                            ALU.mult, ALU.add)                     # c_t
                        nc.scalar.activation(Bt, G2[64:97, :], AF.Tanh)
                        # hLb = 2*hL = tanh(c) * (tanh(og/2) + 1); the 0.5 is
                        # folded into W0recT/W1recT/W2recT
                        nc.vector.scalar_tensor_tensor(
                            hLb, G2[0:33, :], 1.0, Bt, ALU.add, ALU.mult)
                        # loop-2: CfC chain
                        for c in range(NCH):
                            J = slice(c * CH, (c + 1) * CH)
                            Jw = slice(FPAD + c * CH, FPAD + (c + 1) * CH)
                            P0 = pCp.tile([82, CH], F32, tag="P")
                            nc.tensor.matmul(P0, sb["W0recT"], hLb[:, J],
                                             start=True, stop=True)
                            L0t = spp.tile([97, CH], F32, tag="t97")
                            nc.vector.scalar_tensor_tensor(
                                L0t[0:82, :], P0, 1.0, g0in[:, J],
                                ALU.mult, ALU.add)
                            nc.scalar.activation(F0T[0:82, Jw], L0t[0:82, :],
                                                 AF.Tanh)
                            D0 = pDp.tile([INTER, CH], F32, tag="D")
                            nc.tensor.matmul(D0, sb["Cd0"], F0T[0:50, Jw],
                                             start=True, stop=True)
                            nc.vector.tensor_mul(F0T[96:114, Jw],
                                                 F0T[64:82, Jw], D0)
                            P1f = pCp.tile([82, CH], F32, tag="P")
                            P1 = P1f[0:76, :]
                            nc.tensor.matmul(P1, sb["W1comb"], F0T[0:114, Jw],
                                             start=True, stop=False)
                            nc.tensor.matmul(P1, sb["W1recT"], hLb[:, J],
                                             start=False, stop=True)
                            nc.scalar.activation(F1T[0:76, Jw], P1, AF.Tanh,
                                                 bias=sb["bias1"][:, 0:1])
                            D1f = pDp.tile([INTER, CH], F32, tag="D")
                            D1 = D1f[0:COMMAND, :]
                            nc.tensor.matmul(D1, sb["Cd1"], F1T[0:44, Jw],
                                             start=True, stop=True)
                            nc.vector.tensor_mul(F1T[96:108, Jw],
                                                 F1T[64:76, Jw], D1)
                            P2f = pCp.tile([82, CH], F32, tag="P")
                            P2 = P2f[0:67, :]
                            nc.tensor.matmul(P2, sb["W2comb"], F1T[0:108, Jw],
                                             start=True, stop=False)
                            nc.tensor.matmul(P2, sb["W2recT"], hLb[:, J],
                                             start=False, stop=True)
                            nc.scalar.activation(F2T[0:67, Jw], P2, AF.Tanh,
                                                 bias=sb["bias2"][:, 0:1])
                            D2f = pDp.tile([INTER, CH], F32, tag="D")
                            D2 = D2f[0:MOTOR, :]
                            nc.tensor.matmul(D2, sb["Cd2"], F2T[0:35, Jw],
                                             start=True, stop=True)
                            nc.vector.tensor_mul(F2T[96:99, Jw],
                                                 F2T[64:67, Jw], D2)
                            if last:
                                POf = pDp.tile([INTER, CH], F32, tag="D")
                                PO = POf[0:MOTOR, :]
                                nc.tensor.matmul(PO, sb["C2full"],
                                                 F2T[0:99, Jw],
                                                 start=True, stop=True)
                                nc.scalar.activation(ost, PO, AF.Identity)
                                nc.sync.dma_start(out=out_d[:, J], in_=ost)
                        if not last:
                            # zero each lane's LAST col (its h_T is never a
                            # valid h_{t-1}: the next lane's t=0 reads it
                            # shifted and must see h_{-1} = 0); view col 511
                            # of block 0 is the front-pad boundary col.
                            nc.vector.memset(F0v[:, :, BLK - 1:BLK], 0.0)
                            nc.vector.memset(F1v[:, :, BLK - 1:BLK], 0.0)
                            nc.vector.memset(F2v[:, :, BLK - 1:BLK], 0.0)
                    # calibration-only dummy ops (dmm/dfw opts)
                    for _ in range(dmm):
                        dpsf = pGp.tile([97, CH], F32, tag="G")
                        nc.tensor.matmul(dpsf, sb["WHY0"][0:97, :],
                                         zinY[:, 0:CH], start=True, stop=True)
                    for _ in range(dfw):
                        nc.vector.tensor_mul(Bt, SG[64:97, :], SG[64:97, :])
    nc.compile()
    return nc


def host_prep(inputs, T=T_FULL):
    x = np.asarray(inputs["x"], np.float32)
    w = prep_weights(inputs)
    in_maps = []
    for i in range(N_CORES):
        xs = x[i * BS:(i + 1) * BS, :T, :]          # (BS, T, 512)
        xt = np.ascontiguousarray(xs.transpose(2, 0, 1)).reshape(IN_DIM, NCOL)
        xt = xt.reshape(4, 128, NCOL).transpose(1, 0, 2)   # (128, 4, NCOL)
        m = {"xt": np.ascontiguousarray(xt)}
        m.update(w)
        in_maps.append(m)
    return in_maps


def gather_output(results, T=T_FULL):
    outs = []
    for i in range(N_CORES):
        o = np.asarray(results[i]["out"]).reshape(MOTOR, BS, BLK)
        outs.append(o.transpose(1, 2, 0))            # (BS, T, 3)
    return np.concatenate(outs, axis=0)


_PROGRAM_CACHE = {}


def kernel(**inputs):
    T = T_FULL
    if T not in _PROGRAM_CACHE:
        _PROGRAM_CACHE[T] = build_program(T)
    nc = _PROGRAM_CACHE[T]
    in_maps = host_prep(inputs, T)
    res = run_bass_kernel_spmd(nc, in_maps, list(range(N_CORES)))
    return gather_output(res.results, T)


# revision 8
# speedup vs baseline: 3.9214x; 1.0827x over previous
"""Trainium2 Bass kernel for DinMod LSTM+CfC via parallel-in-time iteration.

Key idea: replace the T=512 sequential scan (~33 instructions/step, ~17k
instructions) with 3 full-trajectory sweeps (~650 instructions total):

  sweep m:  z_t   = zin_t + Wh @ h^{m-1}_{t-1}          (big matmuls, all t)
            a_t   = sigmoid(fg_t + 1); b_t = tanh(ia_t) * sigmoid(ig_t)
            c_t   = a_t * c_{t-1} + b_t                  (EXACT via HW
                                                          tensor_tensor_scan)
            hL_t  = tanh(c_t) * sigmoid(og_t)
            h^m_t = CfC(feats_t, hL_t)                   (big matmuls, all t)

The LSTM c-recurrence is linear given the gates, so it is solved exactly
per sweep by the DVE scan instruction; the remaining h-feedback contracts
~10x per sweep (measured in fp32: sweep 3 -> 1.8e-3, sweep 4 -> 2.2e-4
max-rel vs sequential; tolerance is 2e-2).

Column layout: col = 512*b + t for batch-lane b (8 per core), step t.
The scan resets between lanes by forcing a = sig(fg+1) to 0 at each
lane's t=0 col (c_0 = b_0 exactly since c_{-1} = 0). F-trajectory tiles
carry a 512-col zero front block so the shifted (t-1) reads never go out
of bounds, and each lane's LAST col is re-zeroed between sweeps (h_T is
never a valid h_{t-1}; the next lane's t=0 shifted read must see 0).

Quadrant packing (partition bases 0/32/64/96) as in the sequential
baseline. The execute path's cost is dominated by PE instruction count
(~78us/matmul measured), so precomputed input projections are added on
the vector engine (scalar_tensor_tensor on PSUM) instead of identity
re-inject matmuls, and chunks are exactly one PSUM bank (512 cols).
"""

import numpy as np

import concourse.bass as bass
import concourse.mybir as mybir
from concourse import bacc
from concourse.tile import TileContext
from concourse.bass_utils import run_bass_kernel_spmd

IN_DIM, LATENT = 512, 256
INTER, COMMAND, MOTOR = 18, 12, 3
STATE = INTER + COMMAND + MOTOR  # 33
B, T_FULL, N_CORES = 64, 512, 8
BS = B // N_CORES  # 8

BLK = T_FULL            # 512: per-lane block (t = 0..511, no pad cols)
NCOL = BS * BLK         # 4096
FPAD = BLK              # front zero block width of F tiles
FCOL = FPAD + NCOL      # 4608
CH = 512                # free-dim chunk = one PSUM bank exactly; 8 chunks
NCH = NCOL // CH        # 8
SWEEPS = 3

F32 = mybir.dt.float32
AF = mybir.ActivationFunctionType
ALU = mybir.AluOpType

ia_sl, ig_sl = slice(0, 33), slice(33, 66)
fg_sl, og_sl = slice(66, 99), slice(99, 132)


def prep_weights(inp):
    g = {k: np.asarray(v, np.float32) for k, v in inp.items()}
    w = {}
    fc1_w, fc1_b = g["fc1_w"], g["fc1_b"]
    wi, bi, wh = g["lstm_wi"], g["lstm_bi"], g["lstm_wh"]

    weff = wi @ fc1_w                      # (132, 512)
    beff = wi @ fc1_b + bi                 # (132,)

    def gate_pair(lo, hi, bias_lo_extra=0.0):
        m = np.zeros((IN_DIM, 97), np.float32)
        m[:, 0:33] = weff[lo].T
        m[:, 64:97] = weff[hi].T
        bv = np.zeros((97, 1), np.float32)
        bv[0:33, 0] = beff[lo] + bias_lo_extra
        bv[64:97, 0] = beff[hi]
        return m, bv

    w["WXY"], w["biasY"] = gate_pair(fg_sl, ig_sl, 1.0)   # [sfg | sig]
    # og gate via tanh: sig(x) = 0.5*tanh(x/2) + 0.5; the 0.5 logit factor
    # is folded here, the output 0.5/+0.5 into the hL stt and W*recT scales
    w["WXA"], w["biasA"] = gate_pair(og_sl, ia_sl)        # [og' | tia]
    w["WXA"][:, 0:33] *= 0.5
    w["biasA"][0:33] *= 0.5

    # CfC per-layer masked weights
    w1m, w2m, wab, b1v, b2v, btv = [], [], [], [], [], []
    for l in range(3):
        w1m.append(g[f"ff1w{l}"] * g[f"mask{l}"])
        w2m.append(g[f"ff2w{l}"] * g[f"mask{l}"])
        wab.append(0.5 * (g[f"taw{l}"] + g[f"tbw{l}"]))
        b1v.append(g[f"ff1b{l}"])
        b2v.append(g[f"ff2b{l}"])
        btv.append(0.5 * (g[f"tab{l}"] + g[f"tbb{l}"]))

    # L0 input projection through fc1 (feats never materialized on device)
    wx0 = np.zeros((IN_DIM, 82), np.float32)
    bs0 = np.zeros((82, 1), np.float32)
    for qoff, wm, bb in ((0, w1m[0], b1v[0]), (32, w2m[0], b2v[0]),
                         (64, wab[0], btv[0])):
        wx0[:, qoff:qoff + INTER] = (wm[:, :LATENT] @ fc1_w).T
        bs0[qoff:qoff + INTER, 0] = wm[:, :LATENT] @ fc1_b + bb
    w["WX0"], w["bias0"] = wx0, bs0

    # recurrent (hL) projections into each layer's gate quadrants
    def rec_mat(l, p_lo, k, nout):
        m = np.zeros((STATE, nout), np.float32)
        for qoff, wm in ((0, w1m[l]), (32, w2m[l]), (64, wab[l])):
            m[p_lo:p_lo + k, qoff:qoff + k] = wm[:, -k:].T
        return m

    # 0.5x: the hL buffer holds 2*hL = tanh(c)*(tanh(og/2)+1)
    w["W0recT"] = 0.5 * rec_mat(0, 0, INTER, 82)
    w["W1recT"] = 0.5 * rec_mat(1, INTER, COMMAND, 76)
    w["W2recT"] = 0.5 * rec_mat(2, INTER + COMMAND, MOTOR, 67)

    # comb: previous layer's F tile (f1@0:k, f2@32.., pt@96..) -> this layer
    # hl_prev = 0.5*(f1 + f2 + pt)
    def comb_mat(l, kp, nrows, nout, k):
        m = np.zeros((nrows, nout), np.float32)
        for jj in range(kp):
            for r in (jj, 32 + jj, 96 + jj):
                m[r, 0:k] = 0.5 * w1m[l][:, jj]
                m[r, 32:32 + k] = 0.5 * w2m[l][:, jj]
                m[r, 64:64 + k] = 0.5 * wab[l][:, jj]
        return m

    w["W1comb"] = comb_mat(1, INTER, 114, 76, COMMAND)
    w["W2comb"] = comb_mat(2, COMMAND, 108, 67, MOTOR)

    bias1 = np.zeros((76, 1), np.float32)
    bias1[0:12, 0], bias1[32:44, 0], bias1[64:76, 0] = b1v[1], b2v[1], btv[1]
    w["bias1"] = bias1
    bias2 = np.zeros((67, 1), np.float32)
    bias2[0:3, 0], bias2[32:35, 0], bias2[64:67, 0] = b1v[2], b2v[2], btv[2]
    w["bias2"] = bias2

    # f2 - f1 selectors
    for l, k in ((0, INTER), (1, COMMAND), (2, MOTOR)):
        m = np.zeros((32 + k, k), np.float32)
        for j in range(k):
            m[j, j] = -1.0
            m[32 + j, j] = 1.0
        w[f"Cd{l}"] = m

    # motor output: hl2 = 0.5*(f1 + f2 + pt)
    c2 = np.zeros((99, 3), np.float32)
    for j in range(MOTOR):
        c2[j, j] = 0.5
        c2[32 + j, j] = 0.5
        c2[96 + j, j] = 0.5
    w["C2full"] = c2

    # LSTM recurrent: gates from F tiles (h = concat of hl_l = 0.5*(f1+f2+pt))
    koff = [0, INTER, INTER + COMMAND]
    for nm, lo, hi in (("WHY", fg_sl, ig_sl), ("WHA", og_sl, ia_sl)):
        wlo, whi = wh[lo], wh[hi]          # (33, 33) each
        lo_scale = 0.25 if nm == "WHA" else 0.5   # og' logit is halved
        for l, k in ((0, INTER), (1, COMMAND), (2, MOTOR)):
            nr = [114, 108, 99][l]
            m = np.zeros((nr, 97), np.float32)
            for jj in range(k):
                j = koff[l] + jj
                for r in (jj, 32 + jj, 96 + jj):
                    m[r, 0:33] = lo_scale * wlo[:, j]
                    m[r, 64:97] = 0.5 * whi[:, j]
            w[f"{nm}{l}"] = m
    return w


def _weight_specs():
    return {
        "WXY": (512, 97), "WXA": (512, 97), "WX0": (512, 82),
        "biasY": (97, 1), "biasA": (97, 1), "bias0": (82, 1),
        "W0recT": (33, 82), "W1recT": (33, 76), "W2recT": (33, 67),
        "W1comb": (114, 76), "W2comb": (108, 67),
        "bias1": (76, 1), "bias2": (67, 1),
        "Cd0": (50, 18), "Cd1": (44, 12), "Cd2": (35, 3),
        "C2full": (99, 3),
        "WHY0": (114, 97), "WHY1": (108, 97), "WHY2": (99, 97),
        "WHA0": (114, 97), "WHA1": (108, 97), "WHA2": (99, 97),
    }


def build_program(T=T_FULL, opts=()):
    opts = set(opts)
    reps = 1
    sweeps = SWEEPS
    for o in opts:
        if isinstance(o, str) and o.startswith("reps"):
            reps = int(o[4:])
        if isinstance(o, str) and o.startswith("sweeps"):
            sweeps = int(o[6:])

    dmm = dfw = 0
    for o in opts:
        if isinstance(o, str) and o.startswith("dmm"):
            dmm = int(o[3:])       # dummy small matmuls per rep (calibration)
        if isinstance(o, str) and o.startswith("dfw"):
            dfw = int(o[3:])       # dummy full-width DVE ops per rep

    nc = bacc.Bacc("TRN2")
    xt_d = nc.dram_tensor("xt", [128, 4, NCOL], F32, kind="ExternalInput")
    wd = {}
    for nm, shp in _weight_specs().items():
        wd[nm] = nc.dram_tensor(nm, list(shp), F32, kind="ExternalInput")
    out_d = nc.dram_tensor("out", [MOTOR, NCOL], F32, kind="ExternalOutput")

    with TileContext(nc) as tc:
        with tc.tile_pool(name="wp", bufs=1) as wp, \
             tc.tile_pool(name="dp", bufs=1) as dp:
            sb = {}
            for nm, shp in _weight_specs().items():
                rows, cols = shp
                if rows > 128:
                    nch = (rows + 127) // 128
                    t = wp.tile([128, nch, cols], F32, tag=f"w_{nm}")
                    nc.sync.dma_start(
                        out=t, in_=wd[nm].rearrange("(c p) n -> p c n", p=128))
                else:
                    t = wp.tile([rows, cols], F32, tag=f"w_{nm}")
                    nc.sync.dma_start(out=t, in_=wd[nm][:, :])
                sb[nm] = t

            # persistent trajectory buffers
            zinY = dp.tile([97, NCOL], F32)
            zinA = dp.tile([97, NCOL], F32)
            g0in = dp.tile([82, NCOL], F32)
            SG = dp.tile([97, NCOL], F32)    # [a=sig(fg+1)@0:33 | sig(ig)@64:97]
            G2 = dp.tile([97, NCOL], F32)    # [sig(og)@0:33 | tanh(ia)@64:97,
                                             #  then scan-out c_t @64:97]
            Bt = dp.tile([33, NCOL], F32)    # b-term, then reused for tanh(c)
            hLb = dp.tile([33, NCOL], F32)   # LSTM h_t
            F0T = dp.tile([114, FCOL], F32)  # f1@0:18|f2@32:50|t@64:82|pt@96:114
            F1T = dp.tile([108, FCOL], F32)
            F2T = dp.tile([99, FCOL], F32)
            ost = dp.tile([MOTOR, CH], F32)  # out staging per chunk

            for t_ in (F0T, F1T, F2T):
                nc.vector.memset(t_, 0.0)

            # ---- Phase A: project zinY/zinA/g0in from x (through fc1) ----
            with tc.tile_pool(name="xp", bufs=2) as xp, \
                 tc.tile_pool(name="pa", bufs=1, space="PSUM") as pa:
                for c in range(NCH):
                    J = slice(c * CH, (c + 1) * CH)
                    xt_c = xp.tile([128, 4, CH], F32)
                    nc.sync.dma_start(out=xt_c, in_=xt_d[:, :, J])
                    # each target also emits sweep-0's activated gates so the
                    # first sweep skips its gate loop entirely
                    for tgt, lhs, bnm, rows, g0 in (
                            (zinY, "WXY", "biasY", 97, ("SG", AF.Sigmoid)),
                            (zinA, "WXA", "biasA", 97, ("G2", AF.Tanh)),
                            (g0in, "WX0", "bias0", 82, None)):
                        psf = pa.tile([97, CH], F32, tag="pa")
                        ps = psf[0:rows, :]
                        for k in range(4):
                            nc.tensor.matmul(ps, sb[lhs][:, k, 0:rows],
                                             xt_c[:, k, :],
                                             start=(k == 0), stop=(k == 3))
                        nc.scalar.activation(tgt[:, J], ps, AF.Identity,
                                             bias=sb[bnm][:, 0:1])
                        if g0 is not None:
                            gt = SG if g0[0] == "SG" else G2
                            nc.scalar.activation(gt[:, J], ps, g0[1],
                                                 bias=sb[bnm][:, 0:1])

            SGv = SG.rearrange("p (b c) -> p b c", c=BLK)
            F0v = F0T.rearrange("p (b c) -> p b c", c=BLK)
            F1v = F1T.rearrange("p (b c) -> p b c", c=BLK)
            F2v = F2T.rearrange("p (b c) -> p b c", c=BLK)

            # ---- sweeps ----
            with tc.tile_pool(name="pG", bufs=2, space="PSUM") as pGp, \
                 tc.tile_pool(name="pC", bufs=2, space="PSUM") as pCp, \
                 tc.tile_pool(name="pD", bufs=2, space="PSUM") as pDp, \
                 tc.tile_pool(name="sp", bufs=3) as spp:
                for rep in range(reps):
                    for s in range(sweeps):
                        first = (rep == 0 and s == 0)
                        last = (rep == reps - 1 and s == sweeps - 1)
                        # loop-1: LSTM gates (sweep 0's come from phase A)
                        for c in range(NCH) if not first else ():
                            J = slice(c * CH, (c + 1) * CH)
                            Jm = slice(FPAD - 1 + c * CH, FPAD - 1 + (c + 1) * CH)
                            PY = pGp.tile([97, CH], F32, tag="G")
                            PA = pGp.tile([97, CH], F32, tag="G")
                            for P, zin, r0, r1, r2, tgt, fn in (
                                    (PY, zinY, "WHY0", "WHY1", "WHY2",
                                     SG, AF.Sigmoid),
                                    (PA, zinA, "WHA0", "WHA1", "WHA2",
                                     G2, AF.Tanh)):
                                nc.tensor.matmul(P, sb[r0], F0T[:, Jm],
                                                 start=True, stop=False)
                                nc.tensor.matmul(P, sb[r1], F1T[:, Jm],
                                                 start=False, stop=False)
                                nc.tensor.matmul(P, sb[r2], F2T[:, Jm],
                                                 start=False, stop=True)
                                # add the precomputed input part on the DVE,
                                # keeping the PE free for the next matmul
                                Gt = spp.tile([97, CH], F32, tag="t97")
                                nc.vector.scalar_tensor_tensor(
                                    Gt, P, 1.0, zin[:, J], ALU.mult, ALU.add)
                                nc.scalar.activation(tgt[:, J], Gt, fn)
                        # full-width block: exact c-scan, hL.
                        # a=sig(fg+1) is forced to 0 at each lane's t=0 col,
                        # which makes the scan compute c_0 = b_0 exactly
                        # (c_{-1}=0) and resets state between lanes.
                        nc.vector.memset(SGv[0:33, :, 0:1], 0.0)
                        nc.vector.tensor_mul(Bt, G2[64:97, :], SG[64:97, :])
                        nc.vector.tensor_tensor_scan(
                            G2[64:97, :], SG[0:33, :], Bt, 0.0,
                            ALU.mult, ALU.add)                     # c_t
                        nc.scalar.activation(Bt, G2[64:97, :], AF.Tanh)
                        # hLb = 2*hL = tanh(c) * (tanh(og/2) + 1); the 0.5 is
                        # folded into W0recT/W1recT/W2recT
                        nc.vector.scalar_tensor_tensor(
                            hLb, G2[0:33, :], 1.0, Bt, ALU.add, ALU.mult)
                        # loop-2: CfC chain, software-pipelined by STAGE so
                        # the in-order PE queue never waits on a just-issued
                        # act/vmul: each stage runs across all chunks before
                        # its consumers issue (results ~8 dispatch slots old).
                        def cj(c):
                            return (slice(c * CH, (c + 1) * CH),
                                    slice(FPAD + c * CH, FPAD + (c + 1) * CH))
                        for c in range(NCH):            # S1: layer-0 gates
                            J, Jw = cj(c)
                            P0 = pCp.tile([82, CH], F32, tag="P")
                            nc.tensor.matmul(P0, sb["W0recT"], hLb[:, J],
                                             start=True, stop=True)
                            L0t = spp.tile([97, CH], F32, tag="t97")
                            nc.vector.scalar_tensor_tensor(
                                L0t[0:82, :], P0, 1.0, g0in[:, J],
                                ALU.mult, ALU.add)
                            nc.scalar.activation(F0T[0:82, Jw], L0t[0:82, :],
                                                 AF.Tanh)
                        for c in range(NCH):            # S2: pt0
                            J, Jw = cj(c)
                            D0 = pDp.tile([INTER, CH], F32, tag="D")
                            nc.tensor.matmul(D0, sb["Cd0"], F0T[0:50, Jw],
                                             start=True, stop=True)
                            nc.vector.tensor_mul(F0T[96:114, Jw],
                                                 F0T[64:82, Jw], D0)
                        for c in range(NCH):            # S3: layer-1 gates
                            J, Jw = cj(c)
                            P1f = pCp.tile([82, CH], F32, tag="P")
                            P1 = P1f[0:76, :]
                            nc.tensor.matmul(P1, sb["W1comb"], F0T[0:114, Jw],
                                             start=True, stop=False)
                            nc.tensor.matmul(P1, sb["W1recT"], hLb[:, J],
                                             start=False, stop=True)
                            nc.scalar.activation(F1T[0:76, Jw], P1, AF.Tanh,
                                                 bias=sb["bias1"][:, 0:1])
                        for c in range(NCH):            # S4: pt1
                            J, Jw = cj(c)
                            D1f = pDp.tile([INTER, CH], F32, tag="D")
                            D1 = D1f[0:COMMAND, :]
                            nc.tensor.matmul(D1, sb["Cd1"], F1T[0:44, Jw],
                                             start=True, stop=True)
                            nc.vector.tensor_mul(F1T[96:108, Jw],
                                                 F1T[64:76, Jw], D1)
                        for c in range(NCH):            # S5: layer-2 gates
                            J, Jw = cj(c)
                            P2f = pCp.tile([82, CH], F32, tag="P")
                            P2 = P2f[0:67, :]
                            nc.tensor.matmul(P2, sb["W2comb"], F1T[0:108, Jw],
                                             start=True, stop=False)
                            nc.tensor.matmul(P2, sb["W2recT"], hLb[:, J],
                                             start=False, stop=True)
                            nc.scalar.activation(F2T[0:67, Jw], P2, AF.Tanh,
                                                 bias=sb["bias2"][:, 0:1])
                        for c in range(NCH):            # S6: pt2 (+ output)
                            J, Jw = cj(c)
                            D2f = pDp.tile([INTER, CH], F32, tag="D")
                            D2 = D2f[0:MOTOR, :]
                            nc.tensor.matmul(D2, sb["Cd2"], F2T[0:35, Jw],
                                             start=True, stop=True)
                            nc.vector.tensor_mul(F2T[96:99, Jw],
                                                 F2T[64:67, Jw], D2)
                        if last:
                            for c in range(NCH):
                                J, Jw = cj(c)
                                POf = pDp.tile([INTER, CH], F32, tag="D")
                                PO = POf[0:MOTOR, :]
                                nc.tensor.matmul(PO, sb["C2full"],
                                                 F2T[0:99, Jw],
                                                 start=True, stop=True)
                                nc.scalar.activation(ost, PO, AF.Identity)
                                nc.sync.dma_start(out=out_d[:, J], in_=ost)
                        if not last:
                            # zero each lane's LAST col (its h_T is never a
                            # valid h_{t-1}: the next lane's t=0 reads it
                            # shifted and must see h_{-1} = 0); view col 511
                            # of block 0 is the front-pad boundary col.
                            nc.vector.memset(F0v[:, :, BLK - 1:BLK], 0.0)
                            nc.vector.memset(F1v[:, :, BLK - 1:BLK], 0.0)
                            nc.vector.memset(F2v[:, :, BLK - 1:BLK], 0.0)
                    # calibration-only dummy ops (dmm/dfw opts)
                    for _ in range(dmm):
                        dpsf = pGp.tile([97, CH], F32, tag="G")
                        nc.tensor.matmul(dpsf, sb["WHY0"][0:97, :],
                                         zinY[:, 0:CH], start=True, stop=True)
                    for _ in range(dfw):
                        nc.vector.tensor_mul(Bt, SG[64:97, :], SG[64:97, :])
    nc.compile()
    return nc


def host_prep(inputs, T=T_FULL):
    x = np.asarray(inputs["x"], np.float32)
    w = prep_weights(inputs)
    in_maps = []
    for i in range(N_CORES):
        xs = x[i * BS:(i + 1) * BS, :T, :]          # (BS, T, 512)
        xt = np.ascontiguousarray(xs.transpose(2, 0, 1)).reshape(IN_DIM, NCOL)
        xt = xt.reshape(4, 128, NCOL).transpose(1, 0, 2)   # (128, 4, NCOL)
        m = {"xt": np.ascontiguousarray(xt)}
        m.update(w)
        in_maps.append(m)
    return in_maps


def gather_output(results, T=T_FULL):
    outs = []
    for i in range(N_CORES):
        o = np.asarray(results[i]["out"]).reshape(MOTOR, BS, BLK)
        outs.append(o.transpose(1, 2, 0))            # (BS, T, 3)
    return np.concatenate(outs, axis=0)


_PROGRAM_CACHE = {}


def kernel(**inputs):
    T = T_FULL
    if T not in _PROGRAM_CACHE:
        _PROGRAM_CACHE[T] = build_program(T)
    nc = _PROGRAM_CACHE[T]
    in_maps = host_prep(inputs, T)
    res = run_bass_kernel_spmd(nc, in_maps, list(range(N_CORES)))
    return gather_output(res.results, T)
